# revision 1
# baseline (speedup 1.0000x reference)
"""Trainium2 Bass kernel for a dense transformer attention block.

Reference computation (B=2, T=2048, D=1024, H=16, Dh=64, D_FF=4096):
    h   = rmsnorm(x, w_ln1);  qkv = h @ w_qkv.T;  q,k = rope(q,k)
    att = softmax(causal(q k^T / sqrt(Dh)));  h = att @ v
    x   = x + h @ w_out.T
    h   = rmsnorm(x, w_ln2)
    x   = x + (silu(h @ w_gate.T) * (h @ w_up.T)) @ w_down.T

Distribution over 8 NeuronCores, two SPMD launches, no device collectives:
  Phase 1 (core = batch b x head-group hg, 4 heads each): ln1 + QKV + RoPE +
    full causal attention for its 4 heads over its batch, feature-major
    layout throughout.  Softmax without max-subtraction (|score| <=
    ||q||*||k||/8 ~ 8 << 88, so exp cannot overflow); the row sums come from
    a 65th all-ones column appended to V.  Output: normalized h_att^T slice
    [256, 2048] (this is already the feature-major layout phase 2 needs).
  Host: reassemble h_att^T by concatenation + column slicing (no transposes).
  Phase 2 (core = one 512-token chunk): out_proj + residual + ln2 + SwiGLU
    MLP with full weights (streamed), perfectly balanced across cores.

All matmul inputs are bf16 (fp32 PSUM accumulation); residuals are fp32.
ln weights are folded into the adjacent matmul weights on the host.
"""

import os

import numpy as np
import ml_dtypes

import concourse.bass as bass
import concourse.mybir as mybir
import concourse.tile as tile
from concourse import bacc
from concourse.bass_utils import run_bass_kernel_spmd

F32 = mybir.dt.float32
BF16 = mybir.dt.bfloat16
AF = mybir.ActivationFunctionType

B, T, D, DFF, H, DH = 2, 2048, 1024, 4096, 16, 64
HG = 4          # heads per phase-1 core
TOK2 = 512      # tokens per phase-2 core
N_CORES = 8
EPS = 1e-6

bf16 = ml_dtypes.bfloat16


# --------------------------------------------------------------------------
# Phase 1: ln1 + QKV + RoPE + causal attention (per core: one batch, 4 heads)
# --------------------------------------------------------------------------
def build_phase1(T_=T):
    KT = D // 128          # 8 feature k-tiles
    NTT = T_ // 512        # token tiles of 512
    NTB = T_ // 128        # token blocks of 128
    nc = bacc.Bacc(None, target_bir_lowering=False, debug=False)

    xT = nc.dram_tensor("xT", [KT, 128, T_], BF16, kind="ExternalInput")
    wq = nc.dram_tensor("wq", [4, 128, KT * 128], BF16, kind="ExternalInput")
    wv = nc.dram_tensor("wv", [128, KT, HG * DH], BF16, kind="ExternalInput")
    cosq = nc.dram_tensor("cosq", [128, T_], BF16, kind="ExternalInput")
    sinq = nc.dram_tensor("sinq", [128, T_], BF16, kind="ExternalInput")
    cosk = nc.dram_tensor("cosk", [128, T_], BF16, kind="ExternalInput")
    sink = nc.dram_tensor("sink", [128, T_], BF16, kind="ExternalInput")
    tri = nc.dram_tensor("tri", [128, 128], BF16, kind="ExternalInput")
    oT = nc.dram_tensor("oT", [HG, DH + 1, T_], BF16, kind="ExternalOutput")

    with tile.TileContext(nc) as tc:
        with (
            tc.tile_pool(name="big", bufs=1) as big,
            tc.tile_pool(name="qk", bufs=1) as qkp,
            tc.tile_pool(name="vtk", bufs=1) as vtk,
            tc.tile_pool(name="small", bufs=1) as small,
            tc.tile_pool(name="tmp", bufs=3) as tmp,
            tc.tile_pool(name="rope", bufs=2) as ropep,
            tc.tile_pool(name="pt", bufs=4) as ptp,
            tc.tile_pool(name="on", bufs=3) as onp,
            tc.tile_pool(name="psA", bufs=2, space="PSUM") as psA,
            tc.tile_pool(name="psO", bufs=4, space="PSUM") as psO,
        ):
            ones = small.tile([128, 128], BF16, tag="ones")
            nc.vector.memset(ones, 1.0)
            eps_t = small.tile([128, 1], F32, tag="eps")
            nc.vector.memset(eps_t, EPS)
            # x tiles first (they gate the whole front), then weights/tables
            xt, sq = [], []
            for i in range(KT):
                t = big.tile([128, T_], BF16, tag=f"xt{i}", name=f"xt{i}")
                nc.sync.dma_start(out=t, in_=xT[i])
                xt.append(t)
                t2 = big.tile([128, T_], BF16, tag=f"sq{i}", name=f"sq{i}")
                nc.vector.tensor_mul(t2, xt[i], xt[i])
                sq.append(t2)
            wq_t = []
            for j in range(4):
                t = small.tile([128, KT * 128], BF16, tag=f"wq{j}")
                nc.sync.dma_start(out=t, in_=wq[j])
                wq_t.append(t)
            wv_t = small.tile([128, KT, HG * DH], BF16, tag="wv")
            nc.sync.dma_start(out=wv_t, in_=wv[:])
            tri_t = small.tile([128, 128], BF16, tag="tri")
            nc.sync.dma_start(out=tri_t, in_=tri[:])
            ctabs = {}
            for nm, dram in (("cosq", cosq), ("sinq", sinq),
                             ("cosk", cosk), ("sink", sink)):
                t = small.tile([128, T_], BF16, tag=nm, name=nm)
                nc.sync.dma_start(out=t, in_=dram[:])
                ctabs[nm] = t

            # rmsnorm: partition-reduce, rsqrt = exp(-0.5*ln(mean+eps))
            lnt = big.tile([128, T_], F32, tag="lnt")
            for tt in range(NTT):
                ps = psA.tile([128, 1024], F32, tag="ps", name="ps")[:, 0:512]
                for k in range(KT):
                    nc.tensor.matmul(ps, ones, sq[k][:, tt * 512:(tt + 1) * 512],
                                     start=(k == 0), stop=(k == KT - 1))
                nc.scalar.activation(lnt[:, tt * 512:(tt + 1) * 512], ps,
                                     AF.Ln, bias=eps_t, scale=1.0 / D)
            sbc = big.tile([128, T_], BF16, tag="sbc")
            nc.scalar.activation(sbc, lnt, AF.Exp, scale=-0.5)
            ht = []
            for i in range(KT):
                t = big.tile([128, T_], BF16, tag=f"ht{i}")
                nc.vector.tensor_mul(t, xt[i], sbc)
                ht.append(t)

            # Q, K projections -> packed 2-head tiles [128, T] + RoPE
            pk = []
            for j in range(4):
                t = qkp.tile([128, T_], BF16, tag=f"pk{j}", name=f"pk{j}")
                pk.append(t)
            for j in range(4):
                for tt in range(NTT):
                    ps = psA.tile([128, 1024], F32, tag="ps", name="ps")[:, 0:512]
                    for k in range(KT):
                        nc.tensor.matmul(
                            ps, wq_t[j][:, k * 128:(k + 1) * 128],
                            ht[k][:, tt * 512:(tt + 1) * 512],
                            start=(k == 0), stop=(k == KT - 1))
                    nc.vector.tensor_copy(
                        out=pk[j][:, tt * 512:(tt + 1) * 512], in_=ps)
            for j in range(4):
                is_q = j < 2
                ct = ctabs["cosq" if is_q else "cosk"]
                st = ctabs["sinq" if is_q else "sink"]
                rot = ropep.tile([128, T_], BF16, tag="rot")
                nc.vector.tensor_copy(out=rot[0:32], in_=pk[j][32:64])
                nc.vector.tensor_copy(out=rot[32:64], in_=pk[j][0:32])
                nc.vector.tensor_copy(out=rot[64:96], in_=pk[j][96:128])
                nc.vector.tensor_copy(out=rot[96:128], in_=pk[j][64:96])
                nc.vector.tensor_mul(rot, rot, st)
                t1 = ropep.tile([128, T_], BF16, tag="rope1")
                nc.vector.tensor_mul(t1, pk[j], ct)
                nc.vector.tensor_add(pk[j], t1, rot)

            # V token-major [128 tok, HG, DH+1] (65th col = ones)
            vt = []
            for tb in range(NTB):
                ps = psA.tile([128, 1024], F32, tag="ps", name="psv")
                for k in range(KT):
                    nc.tensor.matmul(ps[:, 0:HG * DH],
                                     ht[k][:, tb * 128:(tb + 1) * 128],
                                     wv_t[:, k, :],
                                     start=(k == 0), stop=(k == KT - 1))
                t = vtk.tile([128, HG, DH + 1], BF16, tag=f"vt{tb}",
                             name=f"vt{tb}")
                nc.vector.memset(t[:, :, DH:DH + 1], 1.0)
                nc.vector.tensor_copy(
                    out=t[:, :, 0:DH],
                    in_=ps[:, 0:HG * DH].rearrange("p (g d) -> p g d", g=HG))
                vt.append(t)

            # causal attention, head pairs share row groups of the PE array;
            # output is UNNORMALIZED (col 65 = softmax sums; divided in ph2)
            for hp in range(HG // 2):
                qt_j, kt_j = hp, 2 + hp        # packed tile indices
                for qt in range(NTT):
                    nkb = 4 * qt + 4
                    po2 = [psO.tile([DH + 1, 512], F32, tag="po", name=f"po{u}")
                           for u in range(2)]
                    for kb in range(nkb):
                        sub = kb - 4 * qt
                        lo = max(sub, 0) * 128
                        ksl = slice(kb * 128, (kb + 1) * 128)
                        ss = psA.tile([128, 1024], F32, tag="ps", name="ss")
                        pt = ptp.tile([128, 1024], BF16, tag="pt")
                        for u in range(2):
                            psl = slice(64 * u, 64 * u + 64)
                            qsl = slice(qt * 512 + lo, (qt + 1) * 512)
                            nc.tensor.matmul(ss[:, 512 * u + lo:512 * u + 512],
                                             pk[kt_j][psl, ksl],
                                             pk[qt_j][psl, qsl],
                                             start=True, stop=True)
                        if lo == 0:
                            nc.scalar.activation(pt, ss, AF.Exp)
                        else:
                            nc.scalar.activation(pt[:, lo:512], ss[:, lo:512],
                                                 AF.Exp)
                            nc.scalar.activation(pt[:, 512 + lo:1024],
                                                 ss[:, 512 + lo:1024], AF.Exp)
                        if sub >= 0:
                            nc.vector.tensor_mul(
                                pt[:, lo:lo + 128], pt[:, lo:lo + 128], tri_t)
                            nc.vector.tensor_mul(
                                pt[:, 512 + lo:512 + lo + 128],
                                pt[:, 512 + lo:512 + lo + 128], tri_t)
                        for u in range(2):
                            h = 2 * hp + u
                            nc.tensor.matmul(po2[u][:, lo:],
                                             vt[kb][:, h, :],
                                             pt[:, 512 * u + lo:512 * u + 512],
                                             start=(kb == 0),
                                             stop=(kb == nkb - 1))
                    for u in range(2):
                        h = 2 * hp + u
                        ot = onp.tile([DH + 1, 512], BF16, tag="ot")
                        nc.vector.tensor_copy(out=ot, in_=po2[u])
                        nc.sync.dma_start(out=oT[h, :, qt * 512:(qt + 1) * 512],
                                          in_=ot)
    nc.finalize()
    return nc


# --------------------------------------------------------------------------
# Phase 2: out_proj + residual + ln2 + SwiGLU MLP (per core: 512 tokens)
# --------------------------------------------------------------------------
def build_phase2(TOK=TOK2):
    KT = D // 128     # 8
    KF = DFF // 128   # 32
    nc = bacc.Bacc(None, target_bir_lowering=False, debug=False)

    hattT = nc.dram_tensor("hattT", [KT, 128, TOK], BF16, kind="ExternalInput")
    rec_in = nc.dram_tensor("rec", [H, TOK], BF16, kind="ExternalInput")
    e16 = nc.dram_tensor("e16", [H, KT * 128], BF16, kind="ExternalInput")
    xT = nc.dram_tensor("xT", [KT, 128, TOK], F32, kind="ExternalInput")
    wo = nc.dram_tensor("wo", [KT, 128, KT * 128], BF16, kind="ExternalInput")
    wgu = nc.dram_tensor("wgu", [KF, 128, 2 * KT * 128], BF16, kind="ExternalInput")
    wd = nc.dram_tensor("wd", [KT, 128, KF * 128], BF16, kind="ExternalInput")
    yT = nc.dram_tensor("yT", [KT, 128, TOK], F32, kind="ExternalOutput")

    with tile.TileContext(nc) as tc:
        with (
            tc.tile_pool(name="res", bufs=1) as res,
            tc.tile_pool(name="wres", bufs=1) as wres,
            tc.tile_pool(name="wstream", bufs=4) as wstream,
            tc.tile_pool(name="wdstream", bufs=2) as wdstream,
            tc.tile_pool(name="tmp", bufs=3) as tmp,
            tc.tile_pool(name="hm", bufs=1) as hmp,
            tc.tile_pool(name="psA", bufs=2, space="PSUM") as psA,
            tc.tile_pool(name="psB", bufs=2, space="PSUM") as psB,
        ):
            ones = res.tile([128, 128], BF16, tag="ones")
            nc.vector.memset(ones, 1.0)
            eps_t = res.tile([128, 1], F32, tag="eps")
            nc.vector.memset(eps_t, EPS)
            # resident loads
            ha = []
            for i in range(KT):
                t = res.tile([128, TOK], BF16, tag=f"ha{i}")
                nc.sync.dma_start(out=t, in_=hattT[i])
                ha.append(t)
            rec = res.tile([H, TOK], BF16, tag="rec")
            nc.sync.dma_start(out=rec, in_=rec_in[:])
            e16_t = res.tile([H, KT * 128], BF16, tag="e16")
            nc.sync.dma_start(out=e16_t, in_=e16[:])
            for j in range(KT):
                pbc = psA.tile([128, TOK], F32, tag="ps", name="pbc")
                nc.tensor.matmul(pbc, e16_t[:, j * 128:(j + 1) * 128], rec,
                                 start=True, stop=True)
                nc.vector.tensor_mul(ha[j], ha[j], pbc)
            wo_t = []
            for j in range(KT):
                t = wres.tile([128, KT * 128], BF16, tag=f"wo{j}")
                nc.sync.dma_start(out=t, in_=wo[j])
                wo_t.append(t)


            # out_proj + residual
            xr, x1 = [], []
            for j in range(KT):
                ps = psA.tile([128, TOK], F32, tag="ps")
                for k in range(KT):
                    nc.tensor.matmul(ps, wo_t[j][:, k * 128:(k + 1) * 128], ha[k],
                                     start=(k == 0), stop=(k == KT - 1))
                xrt = res.tile([128, TOK], F32, tag=f"xr{j}", name=f"xr{j}")
                nc.sync.dma_start(out=xrt, in_=xT[j])
                xr.append(xrt)
                t = res.tile([128, TOK], F32, tag=f"x1{j}")
                nc.vector.tensor_add(t, ps, xrt)
                x1.append(t)

            # ln2
            sq = []
            for j in range(KT):
                t = res.tile([128, TOK], BF16, tag=f"sq{j}")
                nc.vector.tensor_mul(t, x1[j], x1[j])
                sq.append(t)
            ps = psA.tile([128, TOK], F32, tag="ps")
            for k in range(KT):
                nc.tensor.matmul(ps, ones, sq[k], start=(k == 0), stop=(k == KT - 1))
            sb0 = res.tile([128, TOK], F32, tag="sbc0")
            nc.scalar.activation(sb0, ps, AF.Sqrt, bias=eps_t, scale=1.0 / D)
            sb = res.tile([128, TOK], BF16, tag="sbc")
            with nc.allow_low_precision(reason="bf16 scale"):
                nc.vector.reciprocal(out=sb, in_=sb0)
            h2 = []
            for j in range(KT):
                t = res.tile([128, TOK], BF16, tag=f"h2{j}")
                nc.vector.tensor_mul(t, x1[j], sb)
                h2.append(t)

            # SwiGLU: gate/up streamed
            hm = []
            for jf in range(KF):
                wt = wstream.tile([128, 2 * KT * 128], BF16, tag="wgu")
                nc.sync.dma_start(out=wt, in_=wgu[jf])
                pg = psB.tile([128, TOK], F32)
                pu = psB.tile([128, TOK], F32)
                for k in range(KT):
                    nc.tensor.matmul(pg, wt[:, k * 128:(k + 1) * 128], h2[k],
                                     start=(k == 0), stop=(k == KT - 1))
                for k in range(KT):
                    nc.tensor.matmul(pu, wt[:, (KT + k) * 128:(KT + k + 1) * 128],
                                     h2[k], start=(k == 0), stop=(k == KT - 1))
                sg = tmp.tile([128, TOK], BF16, tag="sg")
                nc.scalar.activation(sg, pg, AF.Silu)
                t = hmp.tile([128, TOK], BF16, tag=f"hm{jf}")
                nc.vector.tensor_mul(t, pu, sg)
                hm.append(t)

            # down + residual (w_down streamed per output j-tile)
            for j in range(KT):
                wdj = wdstream.tile([128, KF * 128], BF16, tag="wdj")
                nc.sync.dma_start(out=wdj, in_=wd[j])
                ps = psA.tile([128, TOK], F32, tag="ps")
                for kf in range(KF):
                    nc.tensor.matmul(ps, wdj[:, kf * 128:(kf + 1) * 128], hm[kf],
                                     start=(kf == 0), stop=(kf == KF - 1))
                t = tmp.tile([128, TOK], F32, tag="yt")
                nc.vector.tensor_add(t, ps, x1[j])
                nc.sync.dma_start(out=yT[j], in_=t)
    nc.finalize()
    return nc


# --------------------------------------------------------------------------
# Host-side data preparation
# --------------------------------------------------------------------------
def _rope_tables(T_, dim, base=10000.0):
    inv = 1.0 / (base ** (np.arange(0, dim, 2, dtype=np.float64) / dim))
    f = np.arange(T_, dtype=np.float64)[:, None] * inv[None, :]
    emb = np.concatenate((f, f), axis=-1)          # [T, dim]
    return np.cos(emb).astype(np.float32), np.sin(emb).astype(np.float32)


def _lhsT_tiles(w_rows, KT):
    """w_rows [M, K] (rows = out cols of the matmul) -> [M//128, 128, K] with
    [j, p, k*128+c] = w_rows[j*128+c, k*128+p]  (lhsT layout per k-tile)."""
    M, K = w_rows.shape
    t = w_rows.T.reshape(KT, 128, M // 128, 128)   # [k, p, j, c]
    t = t.transpose(2, 1, 0, 3).reshape(M // 128, 128, K)     # [j, p, (k c)]
    return np.ascontiguousarray(t)


def prep_phase1_inputs(x, w_ln1, w_qkv):
    w_eff = (w_qkv.astype(np.float64) * w_ln1.astype(np.float64)[None, :]
             ).astype(np.float32)
    cos, sin = _rope_tables(T, DH)
    sgn = np.ones((DH, 1), np.float32)
    sgn[0:DH // 2] = -1.0   # rot(q) = (-x2, x1): sign baked into sin table rows
    cosT = np.tile(cos.T, (2, 1))            # [128, T] two packed heads
    sinT = np.tile(sin.T * sgn, (2, 1))
    scale = DH ** -0.5
    tri = np.triu(np.ones((128, 128), np.float32))   # keep k<=q (p<=c)

    xb = [np.ascontiguousarray(x[b_].T) for b_ in range(B)]
    in_maps = []
    for core in range(N_CORES):
        b_, hg = divmod(core, HG)
        qr = slice(hg * HG * DH, (hg + 1) * HG * DH)
        kr = slice(D + hg * HG * DH, D + (hg + 1) * HG * DH)
        vr = slice(2 * D + hg * HG * DH, 2 * D + (hg + 1) * HG * DH)
        wqk = np.concatenate([w_eff[qr], w_eff[kr]], axis=0)   # [512, 1024]
        wq_h = _lhsT_tiles(wqk, D // 128).astype(bf16)         # [4,128,1024]
        # v as moving operand: [p, k, c] = w_eff[vbase+c, k*128+p]
        wv_h = (w_eff[vr].T.reshape(D // 128, 128, HG * DH)
                .transpose(1, 0, 2)).astype(bf16)              # [128, 8, 256]
        in_maps.append({
            "xT": xb[b_].astype(bf16).reshape(D // 128, 128, T),
            "wq": wq_h,
            "wv": np.ascontiguousarray(wv_h),
            "cosq": (cosT * scale).astype(bf16),
            "sinq": (sinT * scale).astype(bf16),
            "cosk": cosT.astype(bf16),
            "sink": sinT.astype(bf16),
            "tri": tri.astype(bf16),
        })
    return in_maps, xb


def prep_phase2_inputs(res1, xb, w_ln2, w_out, w_gate, w_up, w_down):
    KT, KF = D // 128, DFF // 128
    # assemble UNNORMALIZED h_att^T + per-(head, token) softmax sums
    hatt, hsums = [], []
    for b_ in range(B):
        rows, srows = [], []
        for hg in range(HG):
            o = np.asarray(res1[b_ * HG + hg]["oT"])       # [HG, DH+1, T]
            rows.append(o[:, 0:DH, :].reshape(HG * DH, T))
            srows.append(o[:, DH, :])                      # [HG, T]
        hatt.append(np.concatenate(rows, axis=0))          # bf16 [1024, T]
        hsums.append(np.concatenate(srows, axis=0))        # bf16 [16, T]
    e16_h = np.repeat(np.eye(H, dtype=np.float32), DH, axis=1).astype(bf16)

    wo_h = _lhsT_tiles(w_out.astype(np.float32), KT).astype(bf16)
    w_gate_eff = (w_gate.astype(np.float64) * w_ln2.astype(np.float64)[None, :]
                  ).astype(np.float32)
    w_up_eff = (w_up.astype(np.float64) * w_ln2.astype(np.float64)[None, :]
                ).astype(np.float32)
    wg_h = _lhsT_tiles(w_gate_eff, KT)                         # [32,128,1024]
    wu_h = _lhsT_tiles(w_up_eff, KT)
    wgu_h = np.concatenate([wg_h, wu_h], axis=2).astype(bf16)  # [32,128,2048]
    wd_h = _lhsT_tiles(w_down.astype(np.float32), KF).astype(bf16)
    # [j, p, (kf c)] with wd_h[j, p, kf*128+c] = w_down[j*128+c, kf*128+p]

    in_maps = []
    for core in range(N_CORES):
        b_, qt = divmod(core, T // TOK2)
        sl = slice(qt * TOK2, (qt + 1) * TOK2)
        in_maps.append({
            "hattT": np.ascontiguousarray(hatt[b_][:, sl]).reshape(KT, 128, TOK2),
            "rec": (1.0 / np.ascontiguousarray(hsums[b_][:, sl])
                    .astype(np.float32)).astype(bf16),
            "e16": e16_h,
            "xT": np.ascontiguousarray(xb[b_][:, sl]).astype(np.float32)
                    .reshape(KT, 128, TOK2),
            "wo": wo_h,
            "wgu": wgu_h,
            "wd": wd_h,
        })
    return in_maps


_NC_CACHE = {}


def _get_nc(phase):
    if phase not in _NC_CACHE:
        _NC_CACHE[phase] = build_phase1() if phase == 1 else build_phase2()
    return _NC_CACHE[phase]


def kernel(x, w_ln1, w_qkv, w_out, w_ln2, w_gate, w_up, w_down):
    x = np.asarray(x, np.float32)
    w_ln1 = np.asarray(w_ln1, np.float32)
    w_qkv = np.asarray(w_qkv, np.float32)
    w_out = np.asarray(w_out, np.float32)
    w_ln2 = np.asarray(w_ln2, np.float32)
    w_gate = np.asarray(w_gate, np.float32)
    w_up = np.asarray(w_up, np.float32)
    w_down = np.asarray(w_down, np.float32)

    trace = os.environ.get("KERNEL_TRACE", "1") != "0"
    cores = list(range(N_CORES))

    in1, xb = prep_phase1_inputs(x, w_ln1, w_qkv)
    r1 = run_bass_kernel_spmd(_get_nc(1), in1, cores, trace=trace)

    in2 = prep_phase2_inputs(r1.results, xb, w_ln2, w_out, w_gate, w_up, w_down)
    r2 = run_bass_kernel_spmd(_get_nc(2), in2, cores, trace=trace)

    out = np.empty((B, T, D), np.float32)
    for core in range(N_CORES):
        b_, qt = divmod(core, T // TOK2)
        yt = np.asarray(r2.results[core]["yT"], np.float32).reshape(D, TOK2)
        out[b_, qt * TOK2:(qt + 1) * TOK2, :] = yt.T

    t1 = r1.exec_time_ns or 0
    t2 = r2.exec_time_ns or 0
    if t1 and t2:
        print(f"Phase1 exec: {t1} ns, Phase2 exec: {t2} ns")
        print(f"HW exec time: {t1 + t2} ns")
    return out

